# revision 20
# baseline (speedup 1.0000x reference)
"""Multi-head causal attention with RoPE on 8 TRN2 NeuronCores.

Tensor-parallel over heads: core c computes heads (2c, 2c+1).
  Phase 1: Q^T,K^T (with RoPE) and V projections from host-packed bf16
           x/weights.  Q^T/K^T (bf16, post-RoPE) and V (bf16) are written
           DIRECTLY into persistent SBUF tiles — no DRAM roundtrip, so
           phase 2 needs no loads at all.
  Phase 2: causal attention per (batch, head), scores^T = K^T_blk^T @ Q^T,
           softmax without max-subtraction.  Softmax denominators: exp
           tiles are accumulated elementwise (bf16) on the Vector engine,
           then two ones-matmuls per 512-query chunk reduce the halves
           over the partition axis directly into a broadcast [128,512]
           PSUM tile; 1/x uses the fast custom-DVE approximation (plain
           InstReciprocal costs ~4us fixed).  The denominator / normalize
           / scatter stage of chunk n is emitted inside chunk n+1's first
           key-pair (one-chunk software pipeline) so the PE and the ctx
           PSUM pool never wait on it.
  Phase 3: per-batch AllToAll (2 collectives; the batch-0 one hides under
           batch-1 attention, the batch-1 one under batch-0's Wo)
           redistributes context from head-sharded to sequence-sharded;
           each core applies the FULL Wo to its 256-token slice per
           batch.  Wo is host-packed in gather-slot order (row u=16p+t ->
           [p,t]) so the post-collective gather is a single linear DMA
           (128 contiguous 8KB descriptors) instead of 2048 512B ones.
All bulk inputs are host-packed so each DMA is 128 large descriptors
(DMA trigger time is proportional to descriptor count and blocks the
issuing engine's sequencer).  Sync HWDGE queue: weights/consts, ctx
scatters, gathers, outputs.  Activation HWDGE queue: x chunks.
"""
import ml_dtypes
import numpy as np

import concourse.bass as bass  # noqa: F401  (engine namespaces live on nc)
import concourse.mybir as mybir
import concourse.tile as tile
from concourse import bacc
from concourse import bass_utils

B, T, DM, H, D = 2, 2048, 2048, 16, 128
NCORES = 8
HPC = H // NCORES        # heads per core
DLOC = HPC * D           # local head width (256)
BT = B * T               # 4096 token rows
P = 128
TCH = 512                # free-dim chunk
XCH = 512                # phase-1 token chunk
NKB = DM // P            # 16 contraction blocks
NTB = T // P             # 16 token blocks per batch
NBCH = T // TCH          # 4 token chunks per batch
TSL = T // NCORES        # 256-token output slice per core per batch
SCALE = 1.0 / float(np.sqrt(D))
F32 = mybir.dt.float32
BF16 = mybir.dt.bfloat16
MUL = mybir.AluOpType.mult
ADD = mybir.AluOpType.add
EXP = mybir.ActivationFunctionType.Exp

_nc_cache = None


def _build():
    nc = bacc.Bacc("TRN2", target_bir_lowering=False, debug=False,
                   num_devices=NCORES)
    xt = nc.dram_tensor("xt", [BT // XCH, P, NKB * XCH], BF16, kind="ExternalInput")
    wq = nc.dram_tensor("wq", [P, NKB * DLOC], BF16, kind="ExternalInput")
    wk = nc.dram_tensor("wk", [P, NKB * DLOC], BF16, kind="ExternalInput")
    wv = nc.dram_tensor("wv", [P, NKB * DLOC], BF16, kind="ExternalInput")
    wo = nc.dram_tensor("wo", [P, NKB * DM], BF16, kind="ExternalInput")
    cf = nc.dram_tensor("cf", [P, T], F32, kind="ExternalInput")
    sf = nc.dram_tensor("sf", [P, T], F32, kind="ExternalInput")
    cm = nc.dram_tensor("cm", [P, 4 * TCH], BF16, kind="ExternalInput")
    oneb = nc.dram_tensor("oneb", [P, P], BF16, kind="ExternalInput")
    # out^T slice: [out_cols, b0 slice | b1 slice]
    outT = nc.dram_tensor("out", [DM, B * TSL], F32, kind="ExternalOutput")

    with tile.TileContext(nc) as tc:
        with tc.tile_pool(name="dram", bufs=1, space="DRAM") as dpool, \
             tc.tile_pool(name="persist", bufs=1) as keep:
            # A2A per batch: 8 rank-blocks x [256 local hd, 256 t-slice]
            ctxA_d = [dpool.tile([DM, TSL], BF16, name=f"ctxA{b}") for b in range(B)]
            gout_d = [dpool.tile([DM, TSL], BF16, name=f"gout{b}") for b in range(B)]

            # persistent SBUF: q/k (bf16, post-RoPE, [dims, keys]) and v
            # ([keys, j, d]) for every (batch, local-head) section
            qT_a = [[keep.tile([P, T], BF16, name=f"qTa{b}_{hl}")
                     for hl in range(HPC)] for b in range(B)]
            kT_a = [[keep.tile([P, T], BF16, name=f"kTa{b}_{hl}")
                     for hl in range(HPC)] for b in range(B)]
            v_a = [[keep.tile([P, NTB, D], BF16, name=f"va{b}_{hl}")
                    for hl in range(HPC)] for b in range(B)]
            cm_s = keep.tile([P, 4 * TCH], BF16)
            oneb_s = keep.tile([P, P], BF16)
            bar_in = dpool.tile([8, 4], F32)
            bar_out = dpool.tile([64, 4], F32, addr_space="Shared")
            # start-skew absorber: cores align here while phase 1 computes
            nc.sync.dma_start(bar_in[:], cf.ap()[0:8, 0:4])
            nc.gpsimd.collective_compute(
                "AllGather", mybir.AluOpType.bypass,
                replica_groups=[list(range(NCORES))],
                ins=[bar_in[:].opt()], outs=[bar_out[:].opt()])

            # ---------------- Phase 1: projections + RoPE ----------------
            with tc.tile_pool(name="p1w", bufs=1) as wpool, \
                 tc.tile_pool(name="p1", bufs=2) as pool, \
                 tc.tile_pool(name="psq", bufs=3, space="PSUM") as psqp, \
                 tc.tile_pool(name="psv", bufs=2, space="PSUM") as psvp:
                wq_s = wpool.tile([P, NKB, DLOC], BF16)
                wk_s = wpool.tile([P, NKB, DLOC], BF16)
                wv_s = wpool.tile([P, NKB, DLOC], BF16)
                cf_s = wpool.tile([P, T], F32)
                sf_s = wpool.tile([P, T], F32)
                nc.sync.dma_start(wq_s[:], wq.ap().rearrange("p (kb m) -> p kb m", kb=NKB))

                for i in range(BT // XCH):
                    bb, ic = i // (T // XCH), i % (T // XCH)
                    xt_t = pool.tile([P, NKB, XCH], BF16, tag="xt")
                    nc.scalar.dma_start(
                        xt_t[:], xt.ap()[i].rearrange("p (kb n) -> p kb n", kb=NKB))
                    if i == 0:
                        nc.sync.dma_start(cf_s[:], cf.ap())
                        nc.sync.dma_start(sf_s[:], sf.ap())
                        nc.scalar.dma_start(
                            wk_s[:], wk.ap().rearrange("p (kb m) -> p kb m", kb=NKB))
                        nc.scalar.dma_start(
                            wv_s[:], wv.ap().rearrange("p (kb m) -> p kb m", kb=NKB))
                        nc.sync.dma_start(cm_s[:], cm.ap())
                        nc.sync.dma_start(oneb_s[:], oneb.ap())
                    cs = cf_s[:, ic * XCH:(ic + 1) * XCH]
                    sn = sf_s[:, ic * XCH:(ic + 1) * XCH]
                    for w_s, dst in ((wq_s, qT_a), (wk_s, kT_a)):
                        for m in range(HPC):
                            ps = psqp.tile([P, XCH], F32, tag="qk")
                            for kb in range(NKB):
                                nc.tensor.matmul(
                                    ps[:], w_s[:, kb, m * P:(m + 1) * P],
                                    xt_t[:, kb],
                                    start=(kb == 0), stop=(kb == NKB - 1))
                            # RoPE: rq = q*cos_full + rot(q)*sin_signed,
                            # written straight into the persistent bf16 tile
                            tmp = pool.tile([P, XCH], F32, tag="tmp")
                            tmp2 = pool.tile([P, XCH], F32, tag="tmp2")
                            nc.vector.tensor_tensor(tmp[0:64], ps[64:128], sn[0:64], MUL)
                            nc.vector.tensor_tensor(tmp[64:128], ps[0:64], sn[64:128], MUL)
                            nc.vector.tensor_tensor(tmp2[:], ps[:], cs, MUL)
                            nc.vector.tensor_tensor(
                                dst[bb][m][:, ic * XCH:(ic + 1) * XCH],
                                tmp2[:], tmp[:], ADD)
                    for tb in range(XCH // P):
                        psv = psvp.tile([P, DLOC], F32, tag="v")
                        for kb in range(NKB):
                            nc.tensor.matmul(
                                psv[:], xt_t[:, kb, tb * P:(tb + 1) * P],
                                wv_s[:, kb],
                                start=(kb == 0), stop=(kb == NKB - 1))
                        jx = ic * (XCH // P) + tb
                        nc.vector.tensor_copy(v_a[bb][0][:, jx], psv[:, 0:D])
                        nc.vector.tensor_copy(v_a[bb][1][:, jx], psv[:, D:DLOC])

            # ------------- Phase 2: causal attention + per-batch A2A ---------
            with tc.tile_pool(name="p3w", bufs=1) as wpool3:
                g_ts = [wpool3.tile([P, NKB, TSL], BF16, name=f"g{b}_t")
                        for b in range(B)]
                with tc.tile_pool(name="p2", bufs=3) as pool2, \
                     tc.tile_pool(name="p2s", bufs=2) as spool, \
                     tc.tile_pool(name="p2t", bufs=6) as ppool, \
                     tc.tile_pool(name="ps_s", bufs=2, space="PSUM") as ps_sp, \
                     tc.tile_pool(name="ps_acc", bufs=2, space="PSUM") as ps_accp, \
                     tc.tile_pool(name="ps_bcp", bufs=2, space="PSUM") as ps_bcp:
                    wo_s = wpool3.tile([P, NKB, DM], BF16)
                    nc.sync.dma_start(
                        wo_s[:], wo.ap().rearrange("p (t m) -> p t m", t=NKB))

                    secs = [(b, hl) for b in range(B) for hl in range(HPC)]

                    def finalize(pend):
                        cq, S, ps_ctx, bq, hq = pend
                        ps_bc = ps_bcp.tile([P, TCH], F32, tag="bc")
                        nc.tensor.matmul(ps_bc[:], oneb_s[:], S[:, 0],
                                         start=True, stop=False)
                        nc.tensor.matmul(ps_bc[:], oneb_s[:], S[:, 1],
                                         start=False, stop=True)
                        bc_s = pool2.tile([P, TCH], F32, tag="bc_s")
                        nc.vector.reciprocal_approx_fast(bc_s[:], ps_bc[:])
                        ctx_s = pool2.tile([P, TCH], BF16, tag="ctx")
                        nc.vector.tensor_tensor(ctx_s[:], ps_ctx[:], bc_s[:], MUL)
                        nc.sync.dma_start(
                            ctxA_d[bq]
                            .rearrange("(r q p) n -> q p r n", q=HPC, p=P)
                            [hq, :, 2 * cq:2 * cq + 2],
                            ctx_s.rearrange("p (r n) -> p r n", r=2))

                    pend = None
                    for s, (b, hl) in enumerate(secs):
                        kT_s, v_s, qTF_s = kT_a[b][hl], v_a[b][hl], qT_a[b][hl]
                        first = (s == 0)
                        if s == 3:
                            # batch-0 gather on the otherwise-idle Pool SWDGE
                            # queue: a slow A2A must not head-block the sync
                            # queue's ctx scatters
                            nc.gpsimd.dma_start(
                                g_ts[0][:],
                                gout_d[0][:].rearrange("(p t) n -> p t n", p=P))
                        for cq in range(NBCH):
                            nblk = 4 * cq + 4
                            qT_c = qTF_s[:, cq * TCH:(cq + 1) * TCH]
                            ps_ctx = ps_accp.tile([P, TCH], F32, tag="ctx")
                            S = spool.tile([P, 2, TCH], BF16, tag="S")
                            s_tiles = 0
                            pT0 = None
                            # diagonal pairs first, so the causally-restricted
                            # ctx matmuls can start full-width and stop on a
                            # full-width off-diagonal block
                            if first or cq == 0:
                                pairs = [2 * k for k in range(nblk // 2)]
                            else:
                                pairs = ([4 * cq, 4 * cq + 2]
                                         + [2 * k for k in range(2 * cq)])
                            for jp, j0 in enumerate(pairs):
                                ps_sc = ps_sp.tile([P, 2, TCH], F32, tag="s")
                                vm0 = j0 - 4 * cq
                                for h in range(2):
                                    # diagonal blocks: skip the all-masked
                                    # column prefix (warm PSUM only)
                                    off = ((vm0 + h) * P
                                           if vm0 >= 0 and not first else 0)
                                    nc.tensor.matmul(
                                        ps_sc[:, h, off:],
                                        kT_s[:, (j0 + h) * P:(j0 + h + 1) * P],
                                        qT_c[:, off:], start=True, stop=True)
                                pT = ppool.tile([P, 2, TCH], BF16, tag="pT")
                                vmask = j0 - 4 * cq
                                if vmask >= 0 and not first:
                                    # diagonal pair on warm tiles: skip the
                                    # all-masked column prefix of each block
                                    for h in range(2):
                                        off = (vmask + h) * P
                                        nc.scalar.activation(
                                            pT[:, h, off:], ps_sc[:, h, off:],
                                            EXP, scale=SCALE)
                                else:
                                    nc.scalar.activation(
                                        pT[:], ps_sc[:], EXP, scale=SCALE)
                                if vmask >= 0:
                                    # full-width: ctx matmuls read all columns,
                                    # so every masked entry must be zeroed
                                    nc.vector.tensor_tensor(
                                        pT[:], pT[:],
                                        cm_s[:, vmask * TCH:(vmask + 2) * TCH]
                                        .rearrange("p (v n) -> p v n", v=2), MUL)
                                # softmax-denominator partials, off the PE
                                s_tiles += 1
                                if s_tiles == 1:
                                    pT0 = pT
                                elif s_tiles == 2:
                                    nc.vector.tensor_tensor(S[:], pT0[:], pT[:], ADD)
                                elif vmask >= 0:
                                    c0 = vmask * P
                                    nc.vector.tensor_tensor(
                                        S[:, :, c0:], S[:, :, c0:],
                                        pT[:, :, c0:], ADD)
                                else:
                                    nc.vector.tensor_tensor(S[:], S[:], pT[:], ADD)
                                for h in range(2):
                                    j = j0 + h
                                    coff = ((j - 4 * cq) * P
                                            if not first and cq > 0 and j > 4 * cq
                                            else 0)
                                    nc.tensor.matmul(
                                        ps_ctx[:, coff:], v_s[:, j],
                                        pT[:, h, coff:],
                                        start=(j0 == pairs[0] and h == 0),
                                        stop=(j0 == pairs[-1] and h == 1))
                                if jp == 0 and pend is not None:
                                    finalize(pend)
                                    pend = None
                            pend = (cq, S, ps_ctx, b, hl)
                        if hl == HPC - 1:
                            finalize(pend)
                            pend = None
                            nc.gpsimd.collective_compute(
                                "AllToAll", mybir.AluOpType.bypass,
                                replica_groups=[list(range(NCORES))],
                                ins=[ctxA_d[b][:].opt()],
                                outs=[gout_d[b][:].opt()])

                # ---------------- Phase 3: output projection (full Wo) -----------
                with tc.tile_pool(name="p3", bufs=4) as pool3, \
                     tc.tile_pool(name="ps3", bufs=4, space="PSUM") as ps3:
                    for b in range(B):
                        g_t = g_ts[b]
                        if b == 1:
                            nc.sync.dma_start(
                                g_t[:],
                                gout_d[1][:].rearrange("(p t) n -> p t n", p=P))
                        for m in range(DM // P):
                            pso = ps3.tile([P, TSL], F32, tag="o")
                            for t in range(NKB):
                                nc.tensor.matmul(
                                    pso[:], wo_s[:, t, m * P:(m + 1) * P], g_t[:, t],
                                    start=(t == 0), stop=(t == NKB - 1))
                            o_s = pool3.tile([P, TSL], F32, tag="o_s")
                            nc.scalar.copy(o_s[:], pso[:])
                            nc.sync.dma_start(
                                outT.ap()[m * P:(m + 1) * P, b * TSL:(b + 1) * TSL],
                                o_s[:])

    nc.compile()
    return nc


def _prep_inputs(x, cos, sin, Wq, Wk, Wv, Wo):
    x = np.asarray(x, dtype=np.float32)
    cos = np.asarray(cos, dtype=np.float32)
    sin = np.asarray(sin, dtype=np.float32)
    # xt packed: [chunk, partition, kb*XCH] so each chunk load is 128
    # contiguous 16KB descriptors
    xt2 = np.ascontiguousarray(x.reshape(BT, DM).T)          # [DM, BT]
    xtp = np.ascontiguousarray(
        xt2.reshape(NKB, P, BT // XCH, XCH).transpose(2, 1, 0, 3)
        .reshape(BT // XCH, P, NKB * XCH)).astype(ml_dtypes.bfloat16)
    cf = np.empty((P, T), np.float32)
    cf[:64] = cos.T
    cf[64:] = cos.T
    sf = np.empty((P, T), np.float32)
    sf[:64] = -sin.T
    sf[64:] = sin.T
    qq = np.arange(TCH, dtype=np.int64)[None, :]
    rr = np.arange(P, dtype=np.int64)[:, None]
    cm = np.concatenate(
        [(qq >= v * P + rr).astype(np.float32) for v in range(TCH // P)],
        axis=1).astype(ml_dtypes.bfloat16)
    oneb = np.ones((P, P), np.float32).astype(ml_dtypes.bfloat16)

    def pack_w(w):  # [DM, M] -> [P, NKB*M], contraction-block-major
        w = np.asarray(w, np.float32)
        m = w.shape[1]
        return np.ascontiguousarray(
            w.reshape(NKB, P, m).transpose(1, 0, 2).reshape(P, NKB * m)
        ).astype(ml_dtypes.bfloat16)

    # wo packed in gather-slot order: contraction group t = rows {16p+t},
    # matching the linear [128 x 8KB] gather of the A2A output
    wo_p = np.ascontiguousarray(
        np.asarray(Wo, np.float32).reshape(P, NKB, DM).reshape(P, NKB * DM)
    ).astype(ml_dtypes.bfloat16)
    in_maps = []
    for c in range(NCORES):
        sl = slice(c * DLOC, (c + 1) * DLOC)
        in_maps.append({
            "xt": xtp, "cf": cf, "sf": sf, "cm": cm, "oneb": oneb,
            "wq": pack_w(np.asarray(Wq, np.float32)[:, sl]),
            "wk": pack_w(np.asarray(Wk, np.float32)[:, sl]),
            "wv": pack_w(np.asarray(Wv, np.float32)[:, sl]),
            "wo": wo_p,
        })
    return in_maps


def run(x, mask, cos, sin, Wq, Wk, Wv, Wo, trace=False):
    global _nc_cache
    if _nc_cache is None:
        _nc_cache = _build()
    in_maps = _prep_inputs(x, cos, sin, Wq, Wk, Wv, Wo)
    res = bass_utils.run_bass_kernel_spmd(
        _nc_cache, in_maps, core_ids=list(range(NCORES)), trace=trace)
    out = np.empty((B, T, DM), np.float32)
    for c in range(NCORES):
        o = res.results[c]["out"]  # [DM, B*TSL]
        for b in range(B):
            out[b, c * TSL:(c + 1) * TSL, :] = o[:, b * TSL:(b + 1) * TSL].T
    return out, res


def kernel(x, mask, cos, sin, Wq, Wk, Wv, Wo):
    out, _ = run(x, mask, cos, sin, Wq, Wk, Wv, Wo, trace=False)
    return out


# revision 21
# speedup vs baseline: 1.0360x; 1.0360x over previous
"""Multi-head causal attention with RoPE on 8 TRN2 NeuronCores.

Tensor-parallel over heads: core c computes heads (2c, 2c+1).
  Phase 1: Q^T,K^T (with RoPE) and V projections from host-packed bf16
           x/weights.  Q^T/K^T (bf16, post-RoPE) and V (bf16) are written
           DIRECTLY into persistent SBUF tiles — no DRAM roundtrip, so
           phase 2 needs no loads at all.
  Phase 2: causal attention per (batch, head), scores^T = K^T_blk^T @ Q^T,
           softmax without max-subtraction.  Softmax denominators: exp
           tiles are accumulated elementwise (bf16) on the Vector engine,
           then two ones-matmuls per 512-query chunk reduce the halves
           over the partition axis directly into a broadcast [128,512]
           PSUM tile; 1/x uses the fast custom-DVE approximation (plain
           InstReciprocal costs ~4us fixed).  The denominator / normalize
           / scatter stage of chunk n is emitted inside chunk n+1's first
           key-pair (one-chunk software pipeline) so the PE and the ctx
           PSUM pool never wait on it.
  Phase 3: per-batch AllToAll (2 collectives; the batch-0 one hides under
           batch-1 attention, the batch-1 one under batch-0's Wo)
           redistributes context from head-sharded to sequence-sharded;
           each core applies the FULL Wo to its 256-token slice per
           batch.  Wo is host-packed in gather-slot order (row u=16p+t ->
           [p,t]) so the post-collective gather is a single linear DMA
           (128 contiguous 8KB descriptors) instead of 2048 512B ones.
All bulk inputs are host-packed so each DMA is 128 large descriptors
(DMA trigger time is proportional to descriptor count and blocks the
issuing engine's sequencer).  Sync HWDGE queue: weights/consts, ctx
scatters, gathers, outputs.  Activation HWDGE queue: x chunks.
"""
import ml_dtypes
import numpy as np

import concourse.bass as bass  # noqa: F401  (engine namespaces live on nc)
import concourse.mybir as mybir
import concourse.tile as tile
from concourse import bacc
from concourse import bass_utils

B, T, DM, H, D = 2, 2048, 2048, 16, 128
NCORES = 8
HPC = H // NCORES        # heads per core
DLOC = HPC * D           # local head width (256)
BT = B * T               # 4096 token rows
P = 128
TCH = 512                # free-dim chunk
XCH = 512                # phase-1 token chunk
NKB = DM // P            # 16 contraction blocks
NTB = T // P             # 16 token blocks per batch
NBCH = T // TCH          # 4 token chunks per batch
TSL = T // NCORES        # 256-token output slice per core per batch
SCALE = 1.0 / float(np.sqrt(D))
F32 = mybir.dt.float32
BF16 = mybir.dt.bfloat16
MUL = mybir.AluOpType.mult
ADD = mybir.AluOpType.add
EXP = mybir.ActivationFunctionType.Exp

_nc_cache = None


def _build():
    nc = bacc.Bacc("TRN2", target_bir_lowering=False, debug=False,
                   num_devices=NCORES)
    xt = nc.dram_tensor("xt", [BT // XCH, P, NKB * XCH], BF16, kind="ExternalInput")
    wq = nc.dram_tensor("wq", [P, NKB * DLOC], BF16, kind="ExternalInput")
    wk = nc.dram_tensor("wk", [P, NKB * DLOC], BF16, kind="ExternalInput")
    wv = nc.dram_tensor("wv", [P, NKB * DLOC], BF16, kind="ExternalInput")
    wo = nc.dram_tensor("wo", [P, NKB * DM], BF16, kind="ExternalInput")
    cf = nc.dram_tensor("cf", [P, T], F32, kind="ExternalInput")
    sf = nc.dram_tensor("sf", [P, T], F32, kind="ExternalInput")
    cm = nc.dram_tensor("cm", [P, 4 * TCH], BF16, kind="ExternalInput")
    oneb = nc.dram_tensor("oneb", [P, P], BF16, kind="ExternalInput")
    # out^T slice: [out_cols, b0 slice | b1 slice]
    outT = nc.dram_tensor("out", [DM, B * TSL], F32, kind="ExternalOutput")

    with tile.TileContext(nc) as tc:
        with tc.tile_pool(name="dram", bufs=1, space="DRAM") as dpool, \
             tc.tile_pool(name="persist", bufs=1) as keep:
            # A2A per batch: 8 rank-blocks x [256 local hd, 256 t-slice]
            ctxA_d = [dpool.tile([DM, TSL], BF16, name=f"ctxA{b}") for b in range(B)]
            gout_d = [dpool.tile([DM, TSL], BF16, name=f"gout{b}") for b in range(B)]

            # persistent SBUF: q/k (bf16, post-RoPE, [dims, keys]) and v
            # ([keys, j, d]) for every (batch, local-head) section
            qT_a = [[keep.tile([P, T], BF16, name=f"qTa{b}_{hl}")
                     for hl in range(HPC)] for b in range(B)]
            kT_a = [[keep.tile([P, T], BF16, name=f"kTa{b}_{hl}")
                     for hl in range(HPC)] for b in range(B)]
            v_a = [[keep.tile([P, NTB, D], BF16, name=f"va{b}_{hl}")
                    for hl in range(HPC)] for b in range(B)]
            cm_s = keep.tile([P, 4 * TCH], BF16)
            oneb_s = keep.tile([P, P], BF16)
            bar_in = dpool.tile([8, 4], F32)
            bar_out = dpool.tile([64, 4], F32, addr_space="Shared")
            # start-skew absorber: cores align here while phase 1 computes
            nc.sync.dma_start(bar_in[:], cf.ap()[0:8, 0:4])
            nc.gpsimd.collective_compute(
                "AllGather", mybir.AluOpType.bypass,
                replica_groups=[list(range(NCORES))],
                ins=[bar_in[:].opt()], outs=[bar_out[:].opt()])

            # ---------------- Phase 1: projections + RoPE ----------------
            with tc.tile_pool(name="p1w", bufs=1) as wpool, \
                 tc.tile_pool(name="p1", bufs=2) as pool, \
                 tc.tile_pool(name="psq", bufs=4, space="PSUM") as psqp, \
                 tc.tile_pool(name="psv", bufs=2, space="PSUM") as psvp:
                wq_s = wpool.tile([P, NKB, DLOC], BF16)
                wk_s = wpool.tile([P, NKB, DLOC], BF16)
                wv_s = wpool.tile([P, NKB, DLOC], BF16)
                cf_s = wpool.tile([P, T], F32)
                sf_s = wpool.tile([P, T], F32)
                nc.sync.dma_start(wq_s[:], wq.ap().rearrange("p (kb m) -> p kb m", kb=NKB))

                for i in range(BT // XCH):
                    bb, ic = i // (T // XCH), i % (T // XCH)
                    xt_t = pool.tile([P, NKB, XCH], BF16, tag="xt")
                    nc.scalar.dma_start(
                        xt_t[:], xt.ap()[i].rearrange("p (kb n) -> p kb n", kb=NKB))
                    if i == 0:
                        nc.sync.dma_start(cf_s[:], cf.ap())
                        nc.sync.dma_start(sf_s[:], sf.ap())
                        nc.scalar.dma_start(
                            wk_s[:], wk.ap().rearrange("p (kb m) -> p kb m", kb=NKB))
                        nc.scalar.dma_start(
                            wv_s[:], wv.ap().rearrange("p (kb m) -> p kb m", kb=NKB))
                        nc.sync.dma_start(cm_s[:], cm.ap())
                        nc.sync.dma_start(oneb_s[:], oneb.ap())
                    cs = cf_s[:, ic * XCH:(ic + 1) * XCH]
                    sn = sf_s[:, ic * XCH:(ic + 1) * XCH]
                    for w_s, dst in ((wq_s, qT_a), (wk_s, kT_a)):
                        for m in range(HPC):
                            ps = psqp.tile([P, XCH], F32, tag="qk")
                            for kb in range(NKB):
                                nc.tensor.matmul(
                                    ps[:], w_s[:, kb, m * P:(m + 1) * P],
                                    xt_t[:, kb],
                                    start=(kb == 0), stop=(kb == NKB - 1))
                            # RoPE: rq = q*cos_full + rot(q)*sin_signed,
                            # written straight into the persistent bf16 tile
                            tmp = pool.tile([P, XCH], F32, tag="tmp")
                            tmp2 = pool.tile([P, XCH], F32, tag="tmp2")
                            nc.vector.tensor_tensor(tmp[0:64], ps[64:128], sn[0:64], MUL)
                            nc.vector.tensor_tensor(tmp[64:128], ps[0:64], sn[64:128], MUL)
                            nc.vector.tensor_tensor(tmp2[:], ps[:], cs, MUL)
                            nc.vector.tensor_tensor(
                                dst[bb][m][:, ic * XCH:(ic + 1) * XCH],
                                tmp2[:], tmp[:], ADD)
                    for tb in range(XCH // P):
                        psv = psvp.tile([P, DLOC], F32, tag="v")
                        for kb in range(NKB):
                            nc.tensor.matmul(
                                psv[:], xt_t[:, kb, tb * P:(tb + 1) * P],
                                wv_s[:, kb],
                                start=(kb == 0), stop=(kb == NKB - 1))
                        jx = ic * (XCH // P) + tb
                        nc.vector.tensor_copy(v_a[bb][0][:, jx], psv[:, 0:D])
                        nc.vector.tensor_copy(v_a[bb][1][:, jx], psv[:, D:DLOC])

            # ------------- Phase 2: causal attention + per-batch A2A ---------
            with tc.tile_pool(name="p3w", bufs=1) as wpool3:
                g_ts = [wpool3.tile([P, NKB, TSL], BF16, name=f"g{b}_t")
                        for b in range(B)]
                with tc.tile_pool(name="p2", bufs=3) as pool2, \
                     tc.tile_pool(name="p2s", bufs=2) as spool, \
                     tc.tile_pool(name="p2t", bufs=6) as ppool, \
                     tc.tile_pool(name="ps_s", bufs=2, space="PSUM") as ps_sp, \
                     tc.tile_pool(name="ps_acc", bufs=2, space="PSUM") as ps_accp, \
                     tc.tile_pool(name="ps_bcp", bufs=2, space="PSUM") as ps_bcp:
                    wo_s = wpool3.tile([P, NKB, DM], BF16)
                    nc.sync.dma_start(
                        wo_s[:], wo.ap().rearrange("p (t m) -> p t m", t=NKB))

                    secs = [(b, hl) for b in range(B) for hl in range(HPC)]

                    def finalize(pend):
                        cq, S, ps_ctx, bq, hq = pend
                        ps_bc = ps_bcp.tile([P, TCH], F32, tag="bc")
                        nc.tensor.matmul(ps_bc[:], oneb_s[:], S[:, 0],
                                         start=True, stop=False)
                        nc.tensor.matmul(ps_bc[:], oneb_s[:], S[:, 1],
                                         start=False, stop=True)
                        bc_s = pool2.tile([P, TCH], F32, tag="bc_s")
                        nc.vector.reciprocal_approx_fast(bc_s[:], ps_bc[:])
                        ctx_s = pool2.tile([P, TCH], BF16, tag="ctx")
                        nc.vector.tensor_tensor(ctx_s[:], ps_ctx[:], bc_s[:], MUL)
                        nc.sync.dma_start(
                            ctxA_d[bq]
                            .rearrange("(r q p) n -> q p r n", q=HPC, p=P)
                            [hq, :, 2 * cq:2 * cq + 2],
                            ctx_s.rearrange("p (r n) -> p r n", r=2))

                    pend = None
                    for s, (b, hl) in enumerate(secs):
                        kT_s, v_s, qTF_s = kT_a[b][hl], v_a[b][hl], qT_a[b][hl]
                        first = (s == 0)
                        if s == 3:
                            # batch-0 gather on the otherwise-idle Pool SWDGE
                            # queue: a slow A2A must not head-block the sync
                            # queue's ctx scatters
                            nc.gpsimd.dma_start(
                                g_ts[0][:],
                                gout_d[0][:].rearrange("(p t) n -> p t n", p=P))
                        for cq in range(NBCH):
                            nblk = 4 * cq + 4
                            qT_c = qTF_s[:, cq * TCH:(cq + 1) * TCH]
                            ps_ctx = ps_accp.tile([P, TCH], F32, tag="ctx")
                            S = spool.tile([P, 2, TCH], BF16, tag="S")
                            s_tiles = 0
                            pT0 = None
                            # diagonal pairs first, so the causally-restricted
                            # ctx matmuls can start full-width and stop on a
                            # full-width off-diagonal block
                            if first or cq == 0:
                                pairs = [2 * k for k in range(nblk // 2)]
                            else:
                                pairs = ([4 * cq, 4 * cq + 2]
                                         + [2 * k for k in range(2 * cq)])
                            for jp, j0 in enumerate(pairs):
                                ps_sc = ps_sp.tile([P, 2, TCH], F32, tag="s")
                                vm0 = j0 - 4 * cq
                                for h in range(2):
                                    # diagonal blocks: skip the all-masked
                                    # column prefix (warm PSUM only)
                                    off = ((vm0 + h) * P
                                           if vm0 >= 0 and not first else 0)
                                    nc.tensor.matmul(
                                        ps_sc[:, h, off:],
                                        kT_s[:, (j0 + h) * P:(j0 + h + 1) * P],
                                        qT_c[:, off:], start=True, stop=True)
                                pT = ppool.tile([P, 2, TCH], BF16, tag="pT")
                                vmask = j0 - 4 * cq
                                if vmask >= 0 and not first:
                                    # diagonal pair on warm tiles: skip the
                                    # all-masked column prefix of each block
                                    for h in range(2):
                                        off = (vmask + h) * P
                                        nc.scalar.activation(
                                            pT[:, h, off:], ps_sc[:, h, off:],
                                            EXP, scale=SCALE)
                                else:
                                    nc.scalar.activation(
                                        pT[:], ps_sc[:], EXP, scale=SCALE)
                                if vmask >= 0:
                                    # full-width: ctx matmuls read all columns,
                                    # so every masked entry must be zeroed
                                    nc.vector.tensor_tensor(
                                        pT[:], pT[:],
                                        cm_s[:, vmask * TCH:(vmask + 2) * TCH]
                                        .rearrange("p (v n) -> p v n", v=2), MUL)
                                # softmax-denominator partials, off the PE
                                s_tiles += 1
                                if s_tiles == 1:
                                    pT0 = pT
                                elif s_tiles == 2:
                                    nc.vector.tensor_tensor(S[:], pT0[:], pT[:], ADD)
                                elif vmask >= 0:
                                    c0 = vmask * P
                                    nc.vector.tensor_tensor(
                                        S[:, :, c0:], S[:, :, c0:],
                                        pT[:, :, c0:], ADD)
                                else:
                                    nc.vector.tensor_tensor(S[:], S[:], pT[:], ADD)
                                for h in range(2):
                                    j = j0 + h
                                    coff = ((j - 4 * cq) * P
                                            if not first and cq > 0 and j > 4 * cq
                                            else 0)
                                    nc.tensor.matmul(
                                        ps_ctx[:, coff:], v_s[:, j],
                                        pT[:, h, coff:],
                                        start=(j0 == pairs[0] and h == 0),
                                        stop=(j0 == pairs[-1] and h == 1))
                                if jp == 0 and pend is not None:
                                    finalize(pend)
                                    pend = None
                            pend = (cq, S, ps_ctx, b, hl)
                        if hl == HPC - 1:
                            finalize(pend)
                            pend = None
                            nc.gpsimd.collective_compute(
                                "AllToAll", mybir.AluOpType.bypass,
                                replica_groups=[list(range(NCORES))],
                                ins=[ctxA_d[b][:].opt()],
                                outs=[gout_d[b][:].opt()])

                # ---------------- Phase 3: output projection (full Wo) -----------
                with tc.tile_pool(name="p3", bufs=4) as pool3, \
                     tc.tile_pool(name="ps3", bufs=4, space="PSUM") as ps3:
                    for b in range(B):
                        g_t = g_ts[b]
                        if b == 1:
                            nc.sync.dma_start(
                                g_t[:],
                                gout_d[1][:].rearrange("(p t) n -> p t n", p=P))
                        for m in range(DM // P):
                            pso = ps3.tile([P, TSL], F32, tag="o")
                            for t in range(NKB):
                                nc.tensor.matmul(
                                    pso[:], wo_s[:, t, m * P:(m + 1) * P], g_t[:, t],
                                    start=(t == 0), stop=(t == NKB - 1))
                            o_s = pool3.tile([P, TSL], F32, tag="o_s")
                            nc.scalar.copy(o_s[:], pso[:])
                            nc.sync.dma_start(
                                outT.ap()[m * P:(m + 1) * P, b * TSL:(b + 1) * TSL],
                                o_s[:])

    nc.compile()
    return nc


def _prep_inputs(x, cos, sin, Wq, Wk, Wv, Wo):
    x = np.asarray(x, dtype=np.float32)
    cos = np.asarray(cos, dtype=np.float32)
    sin = np.asarray(sin, dtype=np.float32)
    # xt packed: [chunk, partition, kb*XCH] so each chunk load is 128
    # contiguous 16KB descriptors
    xt2 = np.ascontiguousarray(x.reshape(BT, DM).T)          # [DM, BT]
    xtp = np.ascontiguousarray(
        xt2.reshape(NKB, P, BT // XCH, XCH).transpose(2, 1, 0, 3)
        .reshape(BT // XCH, P, NKB * XCH)).astype(ml_dtypes.bfloat16)
    cf = np.empty((P, T), np.float32)
    cf[:64] = cos.T
    cf[64:] = cos.T
    sf = np.empty((P, T), np.float32)
    sf[:64] = -sin.T
    sf[64:] = sin.T
    qq = np.arange(TCH, dtype=np.int64)[None, :]
    rr = np.arange(P, dtype=np.int64)[:, None]
    cm = np.concatenate(
        [(qq >= v * P + rr).astype(np.float32) for v in range(TCH // P)],
        axis=1).astype(ml_dtypes.bfloat16)
    oneb = np.ones((P, P), np.float32).astype(ml_dtypes.bfloat16)

    def pack_w(w):  # [DM, M] -> [P, NKB*M], contraction-block-major
        w = np.asarray(w, np.float32)
        m = w.shape[1]
        return np.ascontiguousarray(
            w.reshape(NKB, P, m).transpose(1, 0, 2).reshape(P, NKB * m)
        ).astype(ml_dtypes.bfloat16)

    # wo packed in gather-slot order: contraction group t = rows {16p+t},
    # matching the linear [128 x 8KB] gather of the A2A output
    wo_p = np.ascontiguousarray(
        np.asarray(Wo, np.float32).reshape(P, NKB, DM).reshape(P, NKB * DM)
    ).astype(ml_dtypes.bfloat16)
    in_maps = []
    for c in range(NCORES):
        sl = slice(c * DLOC, (c + 1) * DLOC)
        in_maps.append({
            "xt": xtp, "cf": cf, "sf": sf, "cm": cm, "oneb": oneb,
            "wq": pack_w(np.asarray(Wq, np.float32)[:, sl]),
            "wk": pack_w(np.asarray(Wk, np.float32)[:, sl]),
            "wv": pack_w(np.asarray(Wv, np.float32)[:, sl]),
            "wo": wo_p,
        })
    return in_maps


def run(x, mask, cos, sin, Wq, Wk, Wv, Wo, trace=False):
    global _nc_cache
    if _nc_cache is None:
        _nc_cache = _build()
    in_maps = _prep_inputs(x, cos, sin, Wq, Wk, Wv, Wo)
    res = bass_utils.run_bass_kernel_spmd(
        _nc_cache, in_maps, core_ids=list(range(NCORES)), trace=trace)
    out = np.empty((B, T, DM), np.float32)
    for c in range(NCORES):
        o = res.results[c]["out"]  # [DM, B*TSL]
        for b in range(B):
            out[b, c * TSL:(c + 1) * TSL, :] = o[:, b * TSL:(b + 1) * TSL].T
    return out, res


def kernel(x, mask, cos, sin, Wq, Wk, Wv, Wo):
    out, _ = run(x, mask, cos, sin, Wq, Wk, Wv, Wo, trace=False)
    return out


# revision 33
# speedup vs baseline: 1.0546x; 1.0180x over previous
"""Multi-head causal attention with RoPE on 8 TRN2 NeuronCores.

Tensor-parallel over heads: core c computes heads (2c, 2c+1).
  Phase 1: Q^T,K^T (with RoPE) and V projections from host-packed bf16
           x/weights.  Q^T/K^T (bf16, post-RoPE) and V (bf16) are written
           DIRECTLY into persistent SBUF tiles — no DRAM roundtrip, so
           phase 2 needs no loads at all.
  Phase 2: causal attention per (batch, head), scores^T = K^T_blk^T @ Q^T,
           softmax without max-subtraction.  Softmax denominators: exp
           tiles are accumulated elementwise (bf16) on the Vector engine,
           then two ones-matmuls per 512-query chunk reduce the halves
           over the partition axis directly into a broadcast [128,512]
           PSUM tile; 1/x uses the fast custom-DVE approximation (plain
           InstReciprocal costs ~4us fixed).  The denominator / normalize
           / scatter stage of chunk n is emitted inside chunk n+1's first
           key-pair (one-chunk software pipeline) so the PE and the ctx
           PSUM pool never wait on it.
  Phase 3: per-batch AllToAll (2 collectives; the batch-0 one hides under
           batch-1 attention, the batch-1 one under batch-0's Wo)
           redistributes context from head-sharded to sequence-sharded;
           each core applies the FULL Wo to its 256-token slice per
           batch.  Wo is host-packed in gather-slot order (row u=16p+t ->
           [p,t]) so the post-collective gather is a single linear DMA
           (128 contiguous 8KB descriptors) instead of 2048 512B ones.
All bulk inputs are host-packed so each DMA is 128 large descriptors
(DMA trigger time is proportional to descriptor count and blocks the
issuing engine's sequencer).  Sync HWDGE queue: weights/consts, ctx
scatters, gathers, outputs.  Activation HWDGE queue: x chunks.
"""
import ml_dtypes
import numpy as np

import concourse.bass as bass  # noqa: F401  (engine namespaces live on nc)
import concourse.mybir as mybir
import concourse.tile as tile
from concourse import bacc
from concourse import bass_utils

B, T, DM, H, D = 2, 2048, 2048, 16, 128
NCORES = 8
HPC = H // NCORES        # heads per core
DLOC = HPC * D           # local head width (256)
BT = B * T               # 4096 token rows
P = 128
TCH = 512                # free-dim chunk
XCH = 512                # phase-1 token chunk
NKB = DM // P            # 16 contraction blocks
NTB = T // P             # 16 token blocks per batch
NBCH = T // TCH          # 4 token chunks per batch
TSL = T // NCORES        # 256-token output slice per core per batch
SCALE = 1.0 / float(np.sqrt(D))
F32 = mybir.dt.float32
BF16 = mybir.dt.bfloat16
MUL = mybir.AluOpType.mult
ADD = mybir.AluOpType.add
EXP = mybir.ActivationFunctionType.Exp

_nc_cache = None


def _build():
    nc = bacc.Bacc("TRN2", target_bir_lowering=False, debug=False,
                   num_devices=NCORES)
    xt = nc.dram_tensor("xt", [BT // XCH, P, NKB * XCH], BF16, kind="ExternalInput")
    wq = nc.dram_tensor("wq", [P, NKB * DLOC], BF16, kind="ExternalInput")
    wk = nc.dram_tensor("wk", [P, NKB * DLOC], BF16, kind="ExternalInput")
    wv = nc.dram_tensor("wv", [P, NKB * DLOC], BF16, kind="ExternalInput")
    wo = nc.dram_tensor("wo", [P, NKB * DM], BF16, kind="ExternalInput")
    cf = nc.dram_tensor("cf", [P, T], F32, kind="ExternalInput")
    sf = nc.dram_tensor("sf", [P, T], F32, kind="ExternalInput")
    cm = nc.dram_tensor("cm", [P, 4 * TCH], BF16, kind="ExternalInput")
    oneb = nc.dram_tensor("oneb", [P, P], BF16, kind="ExternalInput")
    # out^T slice: [out_cols, b0 slice | b1 slice]
    outT = nc.dram_tensor("out", [DM, B * TSL], F32, kind="ExternalOutput")

    with tile.TileContext(nc) as tc:
        with tc.tile_pool(name="dram", bufs=1, space="DRAM") as dpool, \
             tc.tile_pool(name="persist", bufs=1) as keep:
            # A2A per batch: 8 rank-blocks x [256 local hd, 256 t-slice]
            ctxA_d = [dpool.tile([DM, TSL], BF16, name=f"ctxA{b}") for b in range(B)]
            gout_d = [dpool.tile([DM, TSL], BF16, name=f"gout{b}") for b in range(B)]

            # persistent SBUF: q/k (bf16, post-RoPE, [dims, keys]) and v
            # ([keys, j, d]) for every (batch, local-head) section
            qT_a = [[keep.tile([P, T], BF16, name=f"qTa{b}_{hl}")
                     for hl in range(HPC)] for b in range(B)]
            kT_a = [[keep.tile([P, T], BF16, name=f"kTa{b}_{hl}")
                     for hl in range(HPC)] for b in range(B)]
            v_a = [[keep.tile([P, NTB, D], BF16, name=f"va{b}_{hl}")
                    for hl in range(HPC)] for b in range(B)]
            cm_s = keep.tile([P, 4 * TCH], BF16)
            oneb_s = keep.tile([P, P], BF16)
            bar_in = dpool.tile([8, 4], F32)
            bar_out = dpool.tile([64, 4], F32, addr_space="Shared")
            # start-skew absorber: cores align here while phase 1 computes
            nc.sync.dma_start(bar_in[:], cf.ap()[0:8, 0:4])
            nc.gpsimd.collective_compute(
                "AllGather", mybir.AluOpType.bypass,
                replica_groups=[list(range(NCORES))],
                ins=[bar_in[:].opt()], outs=[bar_out[:].opt()])

            # ---------------- Phase 1: projections + RoPE ----------------
            with tc.tile_pool(name="p1w", bufs=1) as wpool, \
                 tc.tile_pool(name="p1", bufs=2) as pool, \
                 tc.tile_pool(name="psq", bufs=4, space="PSUM") as psqp, \
                 tc.tile_pool(name="psv", bufs=3, space="PSUM") as psvp:
                wq_s = wpool.tile([P, NKB, DLOC], BF16)
                wk_s = wpool.tile([P, NKB, DLOC], BF16)
                wv_s = wpool.tile([P, NKB, DLOC], BF16)
                cf_s = wpool.tile([P, T], F32)
                sf_s = wpool.tile([P, T], F32)
                # Pool SWDGE queue wakes earliest after boot — putting the
                # lead-in-critical wq load there shaves ~4us off the start
                nc.gpsimd.dma_start(wq_s[:], wq.ap().rearrange("p (kb m) -> p kb m", kb=NKB))

                for i in range(BT // XCH):
                    bb, ic = i // (T // XCH), i % (T // XCH)
                    xt_t = pool.tile([P, NKB, XCH], BF16, tag="xt")
                    nc.scalar.dma_start(
                        xt_t[:], xt.ap()[i].rearrange("p (kb n) -> p kb n", kb=NKB))
                    if i == 0:
                        nc.sync.dma_start(cf_s[:], cf.ap())
                        nc.sync.dma_start(sf_s[:], sf.ap())
                        nc.scalar.dma_start(
                            wk_s[:], wk.ap().rearrange("p (kb m) -> p kb m", kb=NKB))
                        nc.scalar.dma_start(
                            wv_s[:], wv.ap().rearrange("p (kb m) -> p kb m", kb=NKB))
                        nc.sync.dma_start(cm_s[:], cm.ap())
                        nc.sync.dma_start(oneb_s[:], oneb.ap())
                    cs = cf_s[:, ic * XCH:(ic + 1) * XCH]
                    sn = sf_s[:, ic * XCH:(ic + 1) * XCH]
                    for w_s, dst in ((wq_s, qT_a), (wk_s, kT_a)):
                        for m in range(HPC):
                            ps = psqp.tile([P, XCH], F32, tag="qk")
                            for kb in range(NKB):
                                nc.tensor.matmul(
                                    ps[:], w_s[:, kb, m * P:(m + 1) * P],
                                    xt_t[:, kb],
                                    start=(kb == 0), stop=(kb == NKB - 1))
                            # RoPE: rq = q*cos_full + rot(q)*sin_signed,
                            # written straight into the persistent bf16 tile
                            tmp = pool.tile([P, XCH], F32, tag="tmp")
                            tmp2 = pool.tile([P, XCH], F32, tag="tmp2")
                            nc.vector.tensor_tensor(tmp[0:64], ps[64:128], sn[0:64], MUL)
                            nc.vector.tensor_tensor(tmp[64:128], ps[0:64], sn[64:128], MUL)
                            nc.vector.tensor_tensor(tmp2[:], ps[:], cs, MUL)
                            nc.vector.tensor_tensor(
                                dst[bb][m][:, ic * XCH:(ic + 1) * XCH],
                                tmp2[:], tmp[:], ADD)
                    for tb in range(XCH // P):
                        psv = psvp.tile([P, DLOC], F32, tag="v")
                        for kb in range(NKB):
                            nc.tensor.matmul(
                                psv[:], xt_t[:, kb, tb * P:(tb + 1) * P],
                                wv_s[:, kb],
                                start=(kb == 0), stop=(kb == NKB - 1))
                        jx = ic * (XCH // P) + tb
                        # Scalar engine is idle in phase 1; Vector is the
                        # busiest co-engine (RoPE) — offload the V copies
                        nc.scalar.copy(v_a[bb][0][:, jx], psv[:, 0:D])
                        nc.scalar.copy(v_a[bb][1][:, jx], psv[:, D:DLOC])

            # ------------- Phase 2: causal attention + per-batch A2A ---------
            with tc.tile_pool(name="p3w", bufs=1) as wpool3:
                g_ts = [wpool3.tile([P, NKB, TSL], BF16, name=f"g{b}_t")
                        for b in range(B)]
                with tc.tile_pool(name="p2", bufs=3) as pool2, \
                     tc.tile_pool(name="p2s", bufs=3) as spool, \
                     tc.tile_pool(name="p2t", bufs=8) as ppool, \
                     tc.tile_pool(name="ps_s", bufs=2, space="PSUM") as ps_sp, \
                     tc.tile_pool(name="ps_acc", bufs=2, space="PSUM") as ps_accp, \
                     tc.tile_pool(name="ps_bcp", bufs=2, space="PSUM") as ps_bcp:
                    wo_s = wpool3.tile([P, NKB, DM], BF16)
                    nc.sync.dma_start(
                        wo_s[:], wo.ap().rearrange("p (t m) -> p t m", t=NKB))

                    secs = [(b, hl) for b in range(B) for hl in range(HPC)]

                    def finalize(pend):
                        cq, S, ps_ctx, bq, hq = pend
                        ps_bc = ps_bcp.tile([P, TCH], F32, tag="bc")
                        nc.tensor.matmul(ps_bc[:], oneb_s[:], S[:, 0],
                                         start=True, stop=False)
                        nc.tensor.matmul(ps_bc[:], oneb_s[:], S[:, 1],
                                         start=False, stop=True)
                        bc_s = pool2.tile([P, TCH], F32, tag="bc_s")
                        nc.vector.reciprocal_approx_fast(bc_s[:], ps_bc[:])
                        ctx_s = pool2.tile([P, TCH], BF16, tag="ctx")
                        nc.vector.tensor_tensor(ctx_s[:], ps_ctx[:], bc_s[:], MUL)
                        nc.sync.dma_start(
                            ctxA_d[bq]
                            .rearrange("(r q p) n -> q p r n", q=HPC, p=P)
                            [hq, :, 2 * cq:2 * cq + 2],
                            ctx_s.rearrange("p (r n) -> p r n", r=2))

                    pend = None
                    for s, (b, hl) in enumerate(secs):
                        kT_s, v_s, qTF_s = kT_a[b][hl], v_a[b][hl], qT_a[b][hl]
                        first = (s == 0)
                        if s == 3:
                            # batch-0 gather on the otherwise-idle Pool SWDGE
                            # queue: a slow A2A must not head-block the sync
                            # queue's ctx scatters
                            nc.gpsimd.dma_start(
                                g_ts[0][:],
                                gout_d[0][:].rearrange("(p t) n -> p t n", p=P))
                        for cq in range(NBCH):
                            nblk = 4 * cq + 4
                            qT_c = qTF_s[:, cq * TCH:(cq + 1) * TCH]
                            ps_ctx = ps_accp.tile([P, TCH], F32, tag="ctx")
                            S = spool.tile([P, 2, TCH], BF16, tag="S")
                            s_tiles = 0
                            pT0 = None
                            # diagonal pairs first, so the causally-restricted
                            # ctx matmuls can start full-width and stop on a
                            # full-width off-diagonal block
                            if first or cq == 0:
                                pairs = [2 * k for k in range(nblk // 2)]
                            else:
                                pairs = ([4 * cq, 4 * cq + 2]
                                         + [2 * k for k in range(2 * cq)])
                            for jp, j0 in enumerate(pairs):
                                ps_sc = ps_sp.tile([P, 2, TCH], F32, tag="s")
                                vm0 = j0 - 4 * cq
                                for h in range(2):
                                    # diagonal blocks: skip the all-masked
                                    # column prefix (warm PSUM only)
                                    off = ((vm0 + h) * P
                                           if vm0 >= 0 and not first else 0)
                                    nc.tensor.matmul(
                                        ps_sc[:, h, off:],
                                        kT_s[:, (j0 + h) * P:(j0 + h + 1) * P],
                                        qT_c[:, off:], start=True, stop=True)
                                pT = ppool.tile([P, 2, TCH], BF16, tag="pT")
                                vmask = j0 - 4 * cq
                                if vmask >= 0 and not first:
                                    # diagonal pair on warm tiles: skip the
                                    # all-masked column prefix of each block
                                    for h in range(2):
                                        off = (vmask + h) * P
                                        nc.scalar.activation(
                                            pT[:, h, off:], ps_sc[:, h, off:],
                                            EXP, scale=SCALE)
                                else:
                                    nc.scalar.activation(
                                        pT[:], ps_sc[:], EXP, scale=SCALE)
                                if vmask >= 0:
                                    # full-width: ctx matmuls read all columns,
                                    # so every masked entry must be zeroed
                                    nc.vector.tensor_tensor(
                                        pT[:], pT[:],
                                        cm_s[:, vmask * TCH:(vmask + 2) * TCH]
                                        .rearrange("p (v n) -> p v n", v=2), MUL)
                                # softmax-denominator partials, off the PE
                                s_tiles += 1
                                if s_tiles == 1:
                                    pT0 = pT
                                elif s_tiles == 2:
                                    nc.vector.tensor_tensor(S[:], pT0[:], pT[:], ADD)
                                elif vmask >= 0:
                                    c0 = vmask * P
                                    nc.vector.tensor_tensor(
                                        S[:, :, c0:], S[:, :, c0:],
                                        pT[:, :, c0:], ADD)
                                else:
                                    nc.vector.tensor_tensor(S[:], S[:], pT[:], ADD)
                                for h in range(2):
                                    j = j0 + h
                                    coff = ((j - 4 * cq) * P
                                            if not first and cq > 0 and j > 4 * cq
                                            else 0)
                                    nc.tensor.matmul(
                                        ps_ctx[:, coff:], v_s[:, j],
                                        pT[:, h, coff:],
                                        start=(j0 == pairs[0] and h == 0),
                                        stop=(j0 == pairs[-1] and h == 1))
                                if jp == 0 and pend is not None:
                                    finalize(pend)
                                    pend = None
                            pend = (cq, S, ps_ctx, b, hl)
                        if hl == HPC - 1:
                            finalize(pend)
                            pend = None
                            nc.gpsimd.collective_compute(
                                "AllToAll", mybir.AluOpType.bypass,
                                replica_groups=[list(range(NCORES))],
                                ins=[ctxA_d[b][:].opt()],
                                outs=[gout_d[b][:].opt()])

                # ---------------- Phase 3: output projection (full Wo) -----------
                with tc.tile_pool(name="p3", bufs=4) as pool3, \
                     tc.tile_pool(name="ps3", bufs=4, space="PSUM") as ps3:
                    for b in range(B):
                        g_t = g_ts[b]
                        if b == 1:
                            nc.sync.dma_start(
                                g_t[:],
                                gout_d[1][:].rearrange("(p t) n -> p t n", p=P))
                        for m in range(DM // P):
                            pso = ps3.tile([P, TSL], F32, tag="o")
                            for t in range(NKB):
                                nc.tensor.matmul(
                                    pso[:], wo_s[:, t, m * P:(m + 1) * P], g_t[:, t],
                                    start=(t == 0), stop=(t == NKB - 1))
                            o_s = pool3.tile([P, TSL], F32, tag="o_s")
                            nc.scalar.copy(o_s[:], pso[:])
                            nc.sync.dma_start(
                                outT.ap()[m * P:(m + 1) * P, b * TSL:(b + 1) * TSL],
                                o_s[:])

    nc.compile()
    return nc


def _prep_inputs(x, cos, sin, Wq, Wk, Wv, Wo):
    x = np.asarray(x, dtype=np.float32)
    cos = np.asarray(cos, dtype=np.float32)
    sin = np.asarray(sin, dtype=np.float32)
    # xt packed: [chunk, partition, kb*XCH] so each chunk load is 128
    # contiguous 16KB descriptors
    xt2 = np.ascontiguousarray(x.reshape(BT, DM).T)          # [DM, BT]
    xtp = np.ascontiguousarray(
        xt2.reshape(NKB, P, BT // XCH, XCH).transpose(2, 1, 0, 3)
        .reshape(BT // XCH, P, NKB * XCH)).astype(ml_dtypes.bfloat16)
    cf = np.empty((P, T), np.float32)
    cf[:64] = cos.T
    cf[64:] = cos.T
    sf = np.empty((P, T), np.float32)
    sf[:64] = -sin.T
    sf[64:] = sin.T
    qq = np.arange(TCH, dtype=np.int64)[None, :]
    rr = np.arange(P, dtype=np.int64)[:, None]
    cm = np.concatenate(
        [(qq >= v * P + rr).astype(np.float32) for v in range(TCH // P)],
        axis=1).astype(ml_dtypes.bfloat16)
    oneb = np.ones((P, P), np.float32).astype(ml_dtypes.bfloat16)

    def pack_w(w):  # [DM, M] -> [P, NKB*M], contraction-block-major
        w = np.asarray(w, np.float32)
        m = w.shape[1]
        return np.ascontiguousarray(
            w.reshape(NKB, P, m).transpose(1, 0, 2).reshape(P, NKB * m)
        ).astype(ml_dtypes.bfloat16)

    # wo packed in gather-slot order: contraction group t = rows {16p+t},
    # matching the linear [128 x 8KB] gather of the A2A output
    wo_p = np.ascontiguousarray(
        np.asarray(Wo, np.float32).reshape(P, NKB, DM).reshape(P, NKB * DM)
    ).astype(ml_dtypes.bfloat16)
    in_maps = []
    for c in range(NCORES):
        sl = slice(c * DLOC, (c + 1) * DLOC)
        in_maps.append({
            "xt": xtp, "cf": cf, "sf": sf, "cm": cm, "oneb": oneb,
            "wq": pack_w(np.asarray(Wq, np.float32)[:, sl]),
            "wk": pack_w(np.asarray(Wk, np.float32)[:, sl]),
            "wv": pack_w(np.asarray(Wv, np.float32)[:, sl]),
            "wo": wo_p,
        })
    return in_maps


def run(x, mask, cos, sin, Wq, Wk, Wv, Wo, trace=False):
    global _nc_cache
    if _nc_cache is None:
        _nc_cache = _build()
    in_maps = _prep_inputs(x, cos, sin, Wq, Wk, Wv, Wo)
    res = bass_utils.run_bass_kernel_spmd(
        _nc_cache, in_maps, core_ids=list(range(NCORES)), trace=trace)
    out = np.empty((B, T, DM), np.float32)
    for c in range(NCORES):
        o = res.results[c]["out"]  # [DM, B*TSL]
        for b in range(B):
            out[b, c * TSL:(c + 1) * TSL, :] = o[:, b * TSL:(b + 1) * TSL].T
    return out, res


def kernel(x, mask, cos, sin, Wq, Wk, Wv, Wo):
    out, _ = run(x, mask, cos, sin, Wq, Wk, Wv, Wo, trace=False)
    return out
